# revision 11
# baseline (speedup 1.0000x reference)
"""Multi-head attention (16 heads, d_model=1024) on 8 Trainium2 NeuronCores.

Sharding: data-parallel over batch (B=2 -> 2 groups of 4 cores), tensor-
parallel over heads within each group (4 heads/core).  Each core:
  - projects q/k/v for its 4 heads (column-sliced Wq/Wk/Wv),
  - computes scores twice (both orientations: [i,j] for the softmax/p_attn
    output and [j,i] for the p@v contraction) with exp on the scalar engine,
  - row-sums come free from an appended ones-column in v (p@v matmul),
  - applies its row-slice of Wo, then a ReduceScatter over the 4-core group
    sums the partial outputs, each core keeping 512 output rows.
Host side only shards/transposes inputs and concatenates the outputs.

All matmuls run as float32r (full-rate on TRN2 when the moving dim >= 256).
"""

import numpy as np

NHEADS = 16
HIDDEN = 1024
DK = 64
B = 2
S = 2048
NCORES = 8
GROUPS = [[0, 1, 2, 3], [4, 5, 6, 7]]
NH_LOC = NHEADS // 4       # 4 heads per core
HL = NH_LOC * DK           # 256 local projection features

_BUILT = {}


def build_nc(seq=S):
    import concourse.bass as bass
    import concourse.mybir as mybir
    import concourse.tile as tile
    from concourse.vector_clock import ScopedClock, VectorClock

    f32 = mybir.dt.float32
    f32r = mybir.dt.float32r
    Exp = mybir.ActivationFunctionType.Exp
    add = mybir.AluOpType.add
    mult = mybir.AluOpType.mult

    N_PROCS = 27

    class SplitDrainTileContext(tile.TileContext):
        """walrus in this image caps sync-waits per CTRL pseudo at 1; split the
        tail drain into one drain per live semaphore."""

        def _drain_and_barrier(self, tick_clock, wait_clock):
            gc = tick_clock.global_clock
            ticks = [gc[p] for p in range(N_PROCS)]
            for p in range(N_PROCS):
                if ticks[p] > 0:
                    vec = VectorClock(
                        [ticks[q] if q == p else 0 for q in range(N_PROCS)]
                    )
                    d = self.nc.sync.drain()
                    wait_clock.add_sem_waits(d.ins, ScopedClock({None: vec}))
            self.nc.sync.drain()
            self.nc.all_engine_barrier()
            assert self.sems is not None
            popped = self.nc._tile_sem_poison_stack.pop()
            assert popped is self._sem_poison
            self.nc.clear_and_free_semaphores(list(self.sems.allocated().values()))
            self.nc.all_engine_barrier()

    NIB = seq // 512           # 512-wide i blocks
    NIT = seq // 128           # 128-wide i tiles
    NJT = seq // 128           # 128-wide j tiles
    NKT = HIDDEN // 128        # contraction tiles for projections

    nc = bass.Bass(num_devices=NCORES)

    xq = nc.declare_dram_parameter("xq", [HIDDEN, seq], f32, isOutput=False)
    xk = nc.declare_dram_parameter("xk", [HIDDEN, seq], f32, isOutput=False)
    xv = nc.declare_dram_parameter("xv", [HIDDEN, seq], f32, isOutput=False)
    wq = nc.declare_dram_parameter("wq", [HIDDEN, HL], f32, isOutput=False)
    wk = nc.declare_dram_parameter("wk", [HIDDEN, HL], f32, isOutput=False)
    wv = nc.declare_dram_parameter("wv", [HIDDEN, HL], f32, isOutput=False)
    wo = nc.declare_dram_parameter("wo", [HL, HIDDEN], f32, isOutput=False)
    bq = nc.declare_dram_parameter("bq", [128, 2], f32, isOutput=False)
    bk = nc.declare_dram_parameter("bk", [128, 2], f32, isOutput=False)
    bv = nc.declare_dram_parameter("bv", [128, 2], f32, isOutput=False)
    bo = nc.declare_dram_parameter("bo", [1, HIDDEN], f32, isOutput=False)
    ident_in = nc.declare_dram_parameter("ident", [128, 128], f32, isOutput=False)

    p_out = nc.declare_dram_parameter("p_out", [NH_LOC, seq, seq], f32, isOutput=True)
    out_part = nc.declare_dram_parameter(
        "out_part", [seq // 4, HIDDEN], f32, isOutput=True
    )

    cc_in = nc.dram_tensor("cc_in", [seq, HIDDEN], f32)
    cc_out = nc.dram_tensor("cc_out", [seq // 4, HIDDEN], f32)

    def r(ap):
        return ap

    with SplitDrainTileContext(nc) as tc:
        import contextlib

        with contextlib.ExitStack() as ctx:
            singles = ctx.enter_context(tc.tile_pool(name="singles", bufs=1))
            persist = ctx.enter_context(tc.tile_pool(name="persist", bufs=1))
            xstream = ctx.enter_context(tc.tile_pool(name="xstream", bufs=4))
            e_pool = ctx.enter_context(tc.tile_pool(name="e_pool", bufs=3))
            p_pool = ctx.enter_context(tc.tile_pool(name="p_pool", bufs=3))
            out_pool = ctx.enter_context(tc.tile_pool(name="out_pool", bufs=2))
            small = ctx.enter_context(tc.tile_pool(name="small", bufs=8))
            rb_pool = ctx.enter_context(tc.tile_pool(name="rb_pool", bufs=2))
            dram_sc = ctx.enter_context(tc.tile_pool(name="dram_sc", bufs=2, space="DRAM"))
            pp_mm = ctx.enter_context(
                tc.tile_pool(name="pp_mm", bufs=2, space="PSUM")
            )
            pp_a = ctx.enter_context(tc.tile_pool(name="pp_a", bufs=1, space="PSUM"))
            pp_st = ctx.enter_context(tc.tile_pool(name="pp_st", bufs=2, space="PSUM"))
            pp_xt = ctx.enter_context(tc.tile_pool(name="pp_xt", bufs=1, space="PSUM"))

            # ---- load weights/biases ----
            wq_sb = singles.tile([128, NKT, HL], f32r, tag="wq_sb")
            wk_sb = singles.tile([128, NKT, HL], f32r, tag="wk_sb")
            wv_sb = singles.tile([128, NKT, HL], f32r, tag="wv_sb")
            wo_sb = singles.tile([128, 2, HIDDEN], f32r, tag="wo_sb")
            for dst, src in ((wq_sb, wq), (wk_sb, wk), (wv_sb, wv)):
                nc.sync.dma_start(
                    out=dst, in_=src.rearrange("(t p) m -> p t m", p=128).bitcast(f32r)
                )
            nc.sync.dma_start(out=wo_sb, in_=wo.rearrange("(t p) m -> p t m", p=128).bitcast(f32r))
            bq_sb = singles.tile([128, 2], f32, tag="bq_sb")
            bk_sb = singles.tile([128, 2], f32, tag="bk_sb")
            bv_sb = singles.tile([128, 2], f32, tag="bv_sb")
            nc.sync.dma_start(out=bq_sb, in_=bq[:, :])
            nc.sync.dma_start(out=bk_sb, in_=bk[:, :])
            nc.sync.dma_start(out=bv_sb, in_=bv[:, :])
            bo_bc = singles.tile([128, HIDDEN], f32, tag="bo_bc")
            _boap = bo[:, :]
            nc.sync.dma_start(
                out=bo_bc,
                in_=bass.AP(tensor=_boap.tensor, offset=_boap.offset,
                            ap=[[0, 128], [1, HIDDEN]]),
            )
            ident = singles.tile([128, 128], f32r, tag="ident")
            nc.sync.dma_start(out=ident, in_=ident_in[:, :].bitcast(f32r))

            # persistent activation tensors
            qt_sb = persist.tile([128, 2, seq], f32r, tag="qt_sb")
            kt_sb = persist.tile([128, 2, seq], f32r, tag="kt_sb")
            vt_sb = persist.tile([128, 2, seq], f32r, tag="vt_sb")
            xt_sb = persist.tile([128, 2, seq], f32r, tag="xt_sb")
            v_sb = persist.tile([128, NJT, NH_LOC, DK + 1], f32r, tag="v_sb")
            nc.vector.memset(v_sb[:, :, :, DK : DK + 1].bitcast(f32), 1.0)

            # ---- q/k/v projections (orientation: features on partitions) ----
            for x_in, w_sb, b_sb, dst in (
                (xq, wq_sb, bq_sb, qt_sb),
                (xk, wk_sb, bk_sb, kt_sb),
                (xv, wv_sb, bv_sb, vt_sb),
            ):
                for ib in range(NIB):
                    psums = [pp_mm.tile([128, 512], f32, tag="mm", name=f"pj{ib}_{j}") for j in range(2)]
                    for kt in range(NKT):
                        x_t = xstream.tile([128, 512], f32r, tag="x_t")
                        nc.sync.dma_start(
                            out=x_t,
                            in_=x_in[kt * 128 : (kt + 1) * 128, ib * 512 : ib * 512 + 512].bitcast(f32r),
                        )
                        for jt in range(2):
                            nc.tensor.matmul(
                                psums[jt],
                                r(w_sb[:, kt, jt * 128 : jt * 128 + 128]),
                                r(x_t),
                                start=(kt == 0),
                                stop=(kt == NKT - 1),
                            )
                    for jt in range(2):
                        nc.vector.tensor_scalar_add(
                            dst[:, jt, ib * 512 : ib * 512 + 512],
                            psums[jt],
                            b_sb[:, jt : jt + 1],
                        )

            # ---- transform vT -> v (seq on partitions) with appended ones col ----
            for jt in range(2):
                for it in range(NIT):
                    tp = pp_mm.tile([128, 128], f32r, tag="mm", name=f"tp{jt}_{it}")
                    nc.tensor.matmul(
                        tp,
                        vt_sb[:, jt, it * 128 : it * 128 + 128],
                        ident,
                        is_transpose=True,
                    )
                    # feats 128*jt..+128 = heads 2jt, 2jt+1
                    nc.vector.tensor_copy(
                        out=v_sb[:, it, 2 * jt : 2 * jt + 2, 0:DK],
                        in_=tp.rearrange("p (h d) -> p h d", h=2),
                    )

            # ---- attention ----
            for h in range(NH_LOC):
                po = (h % 2) * 64
                jt2 = h // 2

                def qk(t_sb, lo, hi):
                    return t_sb[po : po + 64, jt2, lo:hi]

                # B-side: scores^T -> exp -> (p@v)^T with ones row
                for ib in range(NIB):
                    xt_ps = pp_xt.tile([128, 512], f32, tag="xt")
                    for jt in range(NJT):
                        st_ps = pp_st.tile([128, 512], f32, tag="st")
                        nc.tensor.matmul(
                            st_ps,
                            r(qk(kt_sb, jt * 128, jt * 128 + 128)),
                            r(qk(qt_sb, ib * 512, ib * 512 + 512)),
                            start=True,
                            stop=True,
                        )
                        e_t = e_pool.tile([128, 512], f32r, tag="e_t")
                        nc.scalar.activation(e_t, st_ps, Exp)
                        nc.tensor.matmul(
                            xt_ps[0 : DK + 1, :],
                            r(v_sb[:, jt, h, :]),
                            r(e_t),
                            start=(jt == 0),
                            stop=(jt == NJT - 1),
                        )
                    recip = small.tile([1, 512], f32, tag="recip")
                    nc.vector.reciprocal(recip, xt_ps[DK : DK + 1, :])
                    rb_d = dram_sc.tile([1, 512], f32, tag="rb_d")
                    nc.sync.dma_start(out=rb_d[:, :], in_=recip)
                    rb = rb_pool.tile([64, 512], f32, tag="rb")
                    _rba = rb_d[:, :]
                    nc.sync.dma_start(
                        out=rb,
                        in_=bass.AP(tensor=_rba.tensor, offset=_rba.offset,
                                    ap=[[0, 64], [1, 512]]),
                    )
                    nc.vector.tensor_tensor(
                        out=xt_sb[po : po + 64, jt2, ib * 512 : ib * 512 + 512],
                        in0=xt_ps[0:DK, :],
                        in1=rb,
                        op=mult,
                    )

                # A-side: scores -> exp (+row-sum accum) -> normalize -> DMA out
                for it in range(NIT):
                    p_t = p_pool.tile([128, seq], f32, tag="p_t")
                    accs = []
                    A_CHUNK = min(1024, seq)
                    for half in range(seq // A_CHUNK):
                        sc_ps = pp_a.tile([128, A_CHUNK], f32, tag="a")
                        for nb in range(A_CHUNK // 512):
                            lo = half * A_CHUNK + nb * 512
                            nc.tensor.matmul(
                                sc_ps[:, nb * 512 : nb * 512 + 512],
                                r(qk(qt_sb, it * 128, it * 128 + 128)),
                                r(qk(kt_sb, lo, lo + 512)),
                                start=True,
                                stop=True,
                            )
                        acc = small.tile([128, 1], f32, tag="acc")
                        nc.scalar.activation(
                            p_t[:, half * A_CHUNK : (half + 1) * A_CHUNK],
                            sc_ps,
                            Exp,
                            accum_out=acc,
                        )
                        accs.append(acc)
                    rs = accs[0]
                    for a2 in accs[1:]:
                        rs2 = small.tile([128, 1], f32, tag="acc")
                        nc.vector.tensor_tensor(rs2, rs, a2, op=add)
                        rs = rs2
                    rcp = small.tile([128, 1], f32, tag="acc")
                    nc.vector.reciprocal(rcp, rs)
                    nc.vector.tensor_scalar_mul(p_t, p_t, rcp)
                    nc.sync.dma_start(
                        out=p_out[h, it * 128 : it * 128 + 128, :], in_=p_t
                    )

            # ---- output projection (partial) ----
            for it in range(NIT):
                out_t = out_pool.tile([128, HIDDEN], f32, tag="out_t")
                for ob in range(2):
                    o_ps = pp_mm.tile([128, 512], f32, tag="mm")
                    for kt in range(2):
                        nc.tensor.matmul(
                            o_ps,
                            r(xt_sb[:, kt, it * 128 : it * 128 + 128]),
                            r(wo_sb[:, kt, ob * 512 : ob * 512 + 512]),
                            start=(kt == 0),
                            stop=(kt == 1),
                        )
                    nc.vector.tensor_tensor(
                        out=out_t[:, ob * 512 : ob * 512 + 512],
                        in0=o_ps,
                        in1=bo_bc[:, ob * 512 : ob * 512 + 512],
                        op=add,
                    )
                nc.sync.dma_start(
                    out=cc_in[it * 128 : it * 128 + 128, :], in_=out_t
                )

    # ---- reduce-scatter within each 4-core group, then write the shard ----
    with (
        nc.Block() as block,
        nc.semaphore("cc_sem") as cc_sem,
        nc.semaphore("dma_sem") as dma_sem,
    ):

        @block.gpsimd
        def _(gpsimd):
            gpsimd.collective_compute(
                "ReduceScatter",
                mybir.AluOpType.add,
                replica_groups=GROUPS,
                ins=[cc_in[:]],
                outs=[cc_out[:]],
            ).then_inc(cc_sem, 1)
            gpsimd.wait_ge(cc_sem, 1)
            gpsimd.dma_start(out=out_part[:, :], in_=cc_out[:, :]).then_inc(
                dma_sem, 16
            )
            gpsimd.wait_ge(dma_sem, 16)

    _split_multiwaits(nc, mybir)
    return nc


def _split_multiwaits(nc, mybir):
    """This image's walrus caps each instruction at ONE sync wait; hoist
    extra waits into standalone same-engine InstNoOp carriers."""
    import bass_rust

    ctr = 0
    for bb in nc.main_func.blocks:
        insts = bb.instructions
        i = 0
        while i < len(insts):
            ins = insts[i]
            si = ins.sync_info
            if si is not None and len(si.on_wait) > 1:
                waits = list(si.on_wait)
                for w in waits[:-1]:
                    nop = mybir.InstNoOp(name=f"mwsplit-{ctr}", ins=[], outs=[])
                    ctr += 1
                    nop.engine = ins.engine
                    nop.sync_info = bass_rust.SyncInfo(on_wait=[w], on_update=[])
                    nc.register_instruction(nop)
                    insts.insert(i, nop)
                    i += 1
                ins.sync_info = bass_rust.SyncInfo(
                    on_wait=[waits[-1]], on_update=list(si.on_update)
                )
            i += 1
    return nc


def shard_inputs(query, key, value, Wq, bq, Wk, bk, Wv, bv, Wo, bo, seq=S):
    """Build the 8 per-core input maps (all host-side numpy)."""
    scale = np.float32(1.0 / np.sqrt(DK))
    in_maps = []
    xT = {}
    for b in range(B):
        xT[b] = tuple(
            np.ascontiguousarray(t[b].T) for t in (query, key, value)
        )
    for c in range(NCORES):
        b, g = c // 4, c % 4
        cols = slice(g * HL, (g + 1) * HL)
        xq_, xk_, xv_ = xT[b]
        m = {
            "xq": xq_,
            "xk": xk_,
            "xv": xv_,
            "wq": np.ascontiguousarray(Wq[cols, :].T * scale),
            "wk": np.ascontiguousarray(Wk[cols, :].T),
            "wv": np.ascontiguousarray(Wv[cols, :].T),
            "wo": np.ascontiguousarray(Wo[:, cols].T),
            "bq": np.ascontiguousarray((bq[cols] * scale).reshape(2, 128).T),
            "bk": np.ascontiguousarray(bk[cols].reshape(2, 128).T),
            "bv": np.ascontiguousarray(bv[cols].reshape(2, 128).T),
            "bo": np.ascontiguousarray((bo / 4.0).reshape(1, HIDDEN)),
            "ident": np.eye(128, dtype=np.float32),
        }
        in_maps.append({k: v.astype(np.float32, copy=False) for k, v in m.items()})
    return in_maps


def unshard_outputs(results, seq=S):
    out = np.empty((B, seq, HIDDEN), np.float32)
    p_attn = np.empty((B, NHEADS, seq, seq), np.float32)
    rows = seq // 4
    for c in range(NCORES):
        b, g = c // 4, c % 4
        p_attn[b, g * NH_LOC : (g + 1) * NH_LOC] = results[c]["p_out"]
        out[b, g * rows : (g + 1) * rows, :] = results[c]["out_part"]
    return out, p_attn


def kernel(query, key, value, Wq, bq, Wk, bk, Wv, bv, Wo, bo, _trace=False):
    from concourse.bass_utils import run_bass_kernel_spmd

    args = [np.asarray(a, dtype=np.float32) for a in
            (query, key, value, Wq, bq, Wk, bk, Wv, bv, Wo, bo)]
    if "nc" not in _BUILT:
        _BUILT["nc"] = build_nc(S)
    in_maps = shard_inputs(*args)
    res = run_bass_kernel_spmd(
        _BUILT["nc"], in_maps, core_ids=list(range(NCORES)), trace=_trace
    )
    out, p_attn = unshard_outputs(res.results)
    if _trace:
        kernel.last_exec_time_ns = res.exec_time_ns
        kernel.last_trace = res.instructions_and_trace
    return out, p_attn


# revision 13
# speedup vs baseline: 1.0606x; 1.0606x over previous
"""Multi-head attention (16 heads, d_model=1024) on 8 Trainium2 NeuronCores.

Sharding: data-parallel over batch (B=2 -> 2 groups of 4 cores), tensor-
parallel over heads within each group (4 heads/core).  Each core:
  - projects q/k/v for its 4 heads (column-sliced Wq/Wk/Wv),
  - computes scores twice (both orientations: [i,j] for the softmax/p_attn
    output and [j,i] for the p@v contraction) with exp on the scalar engine,
  - row-sums come free from an appended ones-column in v (p@v matmul),
  - applies its row-slice of Wo, then a ReduceScatter over the 4-core group
    sums the partial outputs, each core keeping 512 output rows.
Host side only shards/transposes inputs and concatenates the outputs.

All matmuls run as float32r (full-rate on TRN2 when the moving dim >= 256).
"""

import numpy as np

NHEADS = 16
HIDDEN = 1024
DK = 64
B = 2
S = 2048
NCORES = 8
GROUPS = [[0, 1, 2, 3], [4, 5, 6, 7]]
NH_LOC = NHEADS // 4       # 4 heads per core
HL = NH_LOC * DK           # 256 local projection features

_BUILT = {}


def build_nc(seq=S):
    import concourse.bass as bass
    import concourse.mybir as mybir
    import concourse.tile as tile
    from concourse.vector_clock import ScopedClock, VectorClock

    f32 = mybir.dt.float32
    f32r = mybir.dt.float32r
    Exp = mybir.ActivationFunctionType.Exp
    add = mybir.AluOpType.add
    mult = mybir.AluOpType.mult

    N_PROCS = 27

    class SplitDrainTileContext(tile.TileContext):
        """walrus in this image caps sync-waits per CTRL pseudo at 1; split the
        tail drain into one drain per live semaphore."""

        def _drain_and_barrier(self, tick_clock, wait_clock):
            gc = tick_clock.global_clock
            ticks = [gc[p] for p in range(N_PROCS)]
            for p in range(N_PROCS):
                if ticks[p] > 0:
                    vec = VectorClock(
                        [ticks[q] if q == p else 0 for q in range(N_PROCS)]
                    )
                    d = self.nc.sync.drain()
                    wait_clock.add_sem_waits(d.ins, ScopedClock({None: vec}))
            self.nc.sync.drain()
            self.nc.all_engine_barrier()
            assert self.sems is not None
            popped = self.nc._tile_sem_poison_stack.pop()
            assert popped is self._sem_poison
            self.nc.clear_and_free_semaphores(list(self.sems.allocated().values()))
            self.nc.all_engine_barrier()

    NIB = seq // 512           # 512-wide i blocks
    NIT = seq // 128           # 128-wide i tiles
    NJT = seq // 128           # 128-wide j tiles
    NKT = HIDDEN // 128        # contraction tiles for projections

    nc = bass.Bass(num_devices=NCORES)

    xq = nc.declare_dram_parameter("xq", [HIDDEN, seq], f32, isOutput=False)
    xk = nc.declare_dram_parameter("xk", [HIDDEN, seq], f32, isOutput=False)
    xv = nc.declare_dram_parameter("xv", [HIDDEN, seq], f32, isOutput=False)
    wq = nc.declare_dram_parameter("wq", [HIDDEN, HL], f32, isOutput=False)
    wk = nc.declare_dram_parameter("wk", [HIDDEN, HL], f32, isOutput=False)
    wv = nc.declare_dram_parameter("wv", [HIDDEN, HL], f32, isOutput=False)
    wo = nc.declare_dram_parameter("wo", [HL, HIDDEN], f32, isOutput=False)
    bq = nc.declare_dram_parameter("bq", [128, 2], f32, isOutput=False)
    bk = nc.declare_dram_parameter("bk", [128, 2], f32, isOutput=False)
    bv = nc.declare_dram_parameter("bv", [128, 2], f32, isOutput=False)
    bo = nc.declare_dram_parameter("bo", [1, HIDDEN], f32, isOutput=False)
    ident_in = nc.declare_dram_parameter("ident", [128, 128], f32, isOutput=False)

    p_out = nc.declare_dram_parameter("p_out", [NH_LOC, seq, seq], f32, isOutput=True)
    out_part = nc.declare_dram_parameter(
        "out_part", [seq // 4, HIDDEN], f32, isOutput=True
    )

    cc_in = nc.dram_tensor("cc_in", [seq, HIDDEN], f32)
    cc_out = nc.dram_tensor("cc_out", [seq // 4, HIDDEN], f32)

    def r(ap):
        return ap

    with SplitDrainTileContext(nc) as tc:
        import contextlib

        with contextlib.ExitStack() as ctx:
            singles = ctx.enter_context(tc.tile_pool(name="singles", bufs=1))
            persist = ctx.enter_context(tc.tile_pool(name="persist", bufs=1))
            xstream = ctx.enter_context(tc.tile_pool(name="xstream", bufs=4))
            e_pool = ctx.enter_context(tc.tile_pool(name="e_pool", bufs=3))
            p_pool = ctx.enter_context(tc.tile_pool(name="p_pool", bufs=3))
            out_pool = ctx.enter_context(tc.tile_pool(name="out_pool", bufs=2))
            small = ctx.enter_context(tc.tile_pool(name="small", bufs=8))
            rb_pool = ctx.enter_context(tc.tile_pool(name="rb_pool", bufs=2))
            dram_sc = ctx.enter_context(tc.tile_pool(name="dram_sc", bufs=2, space="DRAM"))
            pp_mm = ctx.enter_context(
                tc.tile_pool(name="pp_mm", bufs=1, space="PSUM")
            )
            pp_a = ctx.enter_context(tc.tile_pool(name="pp_a", bufs=2, space="PSUM"))
            pp_st = ctx.enter_context(tc.tile_pool(name="pp_st", bufs=2, space="PSUM"))
            pp_xt = ctx.enter_context(tc.tile_pool(name="pp_xt", bufs=1, space="PSUM"))

            # ---- load weights/biases ----
            wq_sb = singles.tile([128, NKT, HL], f32r, tag="wq_sb")
            wk_sb = singles.tile([128, NKT, HL], f32r, tag="wk_sb")
            wv_sb = singles.tile([128, NKT, HL], f32r, tag="wv_sb")
            wo_sb = singles.tile([128, 2, HIDDEN], f32r, tag="wo_sb")
            for dst, src in ((wq_sb, wq), (wk_sb, wk), (wv_sb, wv)):
                nc.sync.dma_start(
                    out=dst, in_=src.rearrange("(t p) m -> p t m", p=128).bitcast(f32r)
                )
            nc.sync.dma_start(out=wo_sb, in_=wo.rearrange("(t p) m -> p t m", p=128).bitcast(f32r))
            bq_sb = singles.tile([128, 2], f32, tag="bq_sb")
            bk_sb = singles.tile([128, 2], f32, tag="bk_sb")
            bv_sb = singles.tile([128, 2], f32, tag="bv_sb")
            nc.sync.dma_start(out=bq_sb, in_=bq[:, :])
            nc.sync.dma_start(out=bk_sb, in_=bk[:, :])
            nc.sync.dma_start(out=bv_sb, in_=bv[:, :])
            bo_bc = singles.tile([128, HIDDEN], f32, tag="bo_bc")
            _boap = bo[:, :]
            nc.sync.dma_start(
                out=bo_bc,
                in_=bass.AP(tensor=_boap.tensor, offset=_boap.offset,
                            ap=[[0, 128], [1, HIDDEN]]),
            )
            ident = singles.tile([128, 128], f32r, tag="ident")
            nc.sync.dma_start(out=ident, in_=ident_in[:, :].bitcast(f32r))

            # persistent activation tensors
            qt_sb = persist.tile([128, 2, seq], f32r, tag="qt_sb")
            kt_sb = persist.tile([128, 2, seq], f32r, tag="kt_sb")
            vt_sb = persist.tile([128, 2, seq], f32r, tag="vt_sb")
            xt_sb = persist.tile([128, 2, seq], f32r, tag="xt_sb")
            v_sb = persist.tile([128, NJT, NH_LOC, DK + 1], f32r, tag="v_sb")
            nc.vector.memset(v_sb[:, :, :, DK : DK + 1].bitcast(f32), 1.0)

            # ---- q/k/v projections (orientation: features on partitions) ----
            for x_in, w_sb, b_sb, dst in (
                (xq, wq_sb, bq_sb, qt_sb),
                (xk, wk_sb, bk_sb, kt_sb),
                (xv, wv_sb, bv_sb, vt_sb),
            ):
                for ib in range(NIB):
                    psum = pp_a.tile([128, 1024], f32, tag="a", name=f"pj{ib}")
                    for kt in range(NKT):
                        x_t = xstream.tile([128, 512], f32r, tag="x_t")
                        nc.sync.dma_start(
                            out=x_t,
                            in_=x_in[kt * 128 : (kt + 1) * 128, ib * 512 : ib * 512 + 512].bitcast(f32r),
                        )
                        for jt in range(2):
                            nc.tensor.matmul(
                                psum[:, jt * 512 : jt * 512 + 512],
                                r(w_sb[:, kt, jt * 128 : jt * 128 + 128]),
                                r(x_t),
                                start=(kt == 0),
                                stop=(kt == NKT - 1),
                            )
                    for jt in range(2):
                        nc.vector.tensor_scalar_add(
                            dst[:, jt, ib * 512 : ib * 512 + 512],
                            psum[:, jt * 512 : jt * 512 + 512],
                            b_sb[:, jt : jt + 1],
                        )

            # ---- transform vT -> v (seq on partitions) with appended ones col ----
            for jt in range(2):
                for it in range(NIT):
                    tp = pp_mm.tile([128, 128], f32r, tag="mm", name=f"tp{jt}_{it}")
                    nc.tensor.matmul(
                        tp,
                        vt_sb[:, jt, it * 128 : it * 128 + 128],
                        ident,
                        is_transpose=True,
                    )
                    # feats 128*jt..+128 = heads 2jt, 2jt+1
                    nc.vector.tensor_copy(
                        out=v_sb[:, it, 2 * jt : 2 * jt + 2, 0:DK],
                        in_=tp.rearrange("p (h d) -> p h d", h=2),
                    )

            # ---- attention ----
            def qk(t_sb, h, lo, hi):
                return t_sb[(h % 2) * 64 : (h % 2) * 64 + 64, h // 2, lo:hi]

            # B phase (dense PE work): scores^T -> exp -> (p@v)^T per head
            for h in range(NH_LOC):
                for ib in range(NIB):
                    xt_ps = pp_xt.tile([128, 512], f32, tag="xt", name=f"xt{h}_{ib}")
                    for jt in range(NJT):
                        st_ps = pp_st.tile([128, 512], f32, tag="st", name=f"st{h}_{ib}_{jt}")
                        nc.tensor.matmul(
                            st_ps,
                            r(qk(kt_sb, h, jt * 128, jt * 128 + 128)),
                            r(qk(qt_sb, h, ib * 512, ib * 512 + 512)),
                            start=True,
                            stop=True,
                        )
                        e_t = e_pool.tile([128, 512], f32r, tag="e_t")
                        nc.scalar.activation(e_t, st_ps, Exp)
                        nc.tensor.matmul(
                            xt_ps[0 : DK + 1, :],
                            r(v_sb[:, jt, h, :]),
                            r(e_t),
                            start=(jt == 0),
                            stop=(jt == NJT - 1),
                        )
                    recip = small.tile([1, 512], f32, tag="recip")
                    nc.vector.reciprocal(recip, xt_ps[DK : DK + 1, :])
                    rb_d = dram_sc.tile([1, 512], f32, tag="rb_d")
                    nc.sync.dma_start(out=rb_d[:, :], in_=recip)
                    rb = rb_pool.tile([64, 512], f32, tag="rb")
                    _rba = rb_d[:, :]
                    nc.sync.dma_start(
                        out=rb,
                        in_=bass.AP(tensor=_rba.tensor, offset=_rba.offset,
                                    ap=[[0, 64], [1, 512]]),
                    )
                    nc.vector.tensor_tensor(
                        out=xt_sb[(h % 2) * 64 : (h % 2) * 64 + 64, h // 2,
                                  ib * 512 : ib * 512 + 512],
                        in0=xt_ps[0:DK, :],
                        in1=rb,
                        op=mult,
                    )

            # ---- output projection (partial) + reduce-scatter (overlaps A phase) ----
            for it in range(NIT):
                out_t = out_pool.tile([128, HIDDEN], f32, tag="out_t")
                for ob in range(2):
                    o_ps = pp_mm.tile([128, 512], f32, tag="mm", name=f"o{it}_{ob}")
                    for kt in range(2):
                        nc.tensor.matmul(
                            o_ps,
                            r(xt_sb[:, kt, it * 128 : it * 128 + 128]),
                            r(wo_sb[:, kt, ob * 512 : ob * 512 + 512]),
                            start=(kt == 0),
                            stop=(kt == 1),
                        )
                    nc.vector.tensor_tensor(
                        out=out_t[:, ob * 512 : ob * 512 + 512],
                        in0=o_ps,
                        in1=bo_bc[:, ob * 512 : ob * 512 + 512],
                        op=add,
                    )
                nc.sync.dma_start(
                    out=cc_in[it * 128 : it * 128 + 128, :], in_=out_t
                )
            nc.gpsimd.collective_compute(
                "ReduceScatter",
                mybir.AluOpType.add,
                replica_groups=GROUPS,
                ins=[cc_in[:]],
                outs=[cc_out[:]],
            )
            nc.gpsimd.dma_start(out=out_part[:, :], in_=cc_out[:, :])

            # A phase (ACT/DMA bound, overlaps the reduce-scatter):
            # scores -> exp (+row-sum accum) -> normalize -> DMA p_attn out
            for h in range(NH_LOC):
                for it in range(NIT):
                    p_t = p_pool.tile([128, seq], f32, tag="p_t")
                    accs = []
                    A_CHUNK = min(1024, seq)
                    for half in range(seq // A_CHUNK):
                        sc_ps = pp_a.tile([128, A_CHUNK], f32, tag="a", name=f"sc{h}_{it}_{half}")
                        for nb in range(A_CHUNK // 512):
                            lo = half * A_CHUNK + nb * 512
                            nc.tensor.matmul(
                                sc_ps[:, nb * 512 : nb * 512 + 512],
                                r(qk(qt_sb, h, it * 128, it * 128 + 128)),
                                r(qk(kt_sb, h, lo, lo + 512)),
                                start=True,
                                stop=True,
                            )
                        acc = small.tile([128, 1], f32, tag="acc")
                        nc.scalar.activation(
                            p_t[:, half * A_CHUNK : (half + 1) * A_CHUNK],
                            sc_ps,
                            Exp,
                            accum_out=acc,
                        )
                        accs.append(acc)
                    rs = accs[0]
                    for a2 in accs[1:]:
                        rs2 = small.tile([128, 1], f32, tag="acc")
                        nc.vector.tensor_tensor(rs2, rs, a2, op=add)
                        rs = rs2
                    rcp = small.tile([128, 1], f32, tag="acc")
                    nc.vector.reciprocal(rcp, rs)
                    nc.vector.tensor_scalar_mul(p_t, p_t, rcp)
                    nc.sync.dma_start(
                        out=p_out[h, it * 128 : it * 128 + 128, :], in_=p_t
                    )

    _split_multiwaits(nc, mybir)
    return nc


def _split_multiwaits(nc, mybir):
    """This image's walrus caps each instruction at ONE sync wait; hoist
    extra waits into standalone same-engine InstNoOp carriers."""
    import bass_rust

    ctr = 0
    for bb in nc.main_func.blocks:
        insts = bb.instructions
        i = 0
        while i < len(insts):
            ins = insts[i]
            si = ins.sync_info
            if si is not None and len(si.on_wait) > 1:
                waits = list(si.on_wait)
                for w in waits[:-1]:
                    nop = mybir.InstNoOp(name=f"mwsplit-{ctr}", ins=[], outs=[])
                    ctr += 1
                    nop.engine = ins.engine
                    nop.sync_info = bass_rust.SyncInfo(on_wait=[w], on_update=[])
                    nc.register_instruction(nop)
                    insts.insert(i, nop)
                    i += 1
                ins.sync_info = bass_rust.SyncInfo(
                    on_wait=[waits[-1]], on_update=list(si.on_update)
                )
            i += 1
    return nc


def shard_inputs(query, key, value, Wq, bq, Wk, bk, Wv, bv, Wo, bo, seq=S):
    """Build the 8 per-core input maps (all host-side numpy)."""
    scale = np.float32(1.0 / np.sqrt(DK))
    in_maps = []
    xT = {}
    for b in range(B):
        xT[b] = tuple(
            np.ascontiguousarray(t[b].T) for t in (query, key, value)
        )
    for c in range(NCORES):
        b, g = c // 4, c % 4
        cols = slice(g * HL, (g + 1) * HL)
        xq_, xk_, xv_ = xT[b]
        m = {
            "xq": xq_,
            "xk": xk_,
            "xv": xv_,
            "wq": np.ascontiguousarray(Wq[cols, :].T * scale),
            "wk": np.ascontiguousarray(Wk[cols, :].T),
            "wv": np.ascontiguousarray(Wv[cols, :].T),
            "wo": np.ascontiguousarray(Wo[:, cols].T),
            "bq": np.ascontiguousarray((bq[cols] * scale).reshape(2, 128).T),
            "bk": np.ascontiguousarray(bk[cols].reshape(2, 128).T),
            "bv": np.ascontiguousarray(bv[cols].reshape(2, 128).T),
            "bo": np.ascontiguousarray((bo / 4.0).reshape(1, HIDDEN)),
            "ident": np.eye(128, dtype=np.float32),
        }
        in_maps.append({k: v.astype(np.float32, copy=False) for k, v in m.items()})
    return in_maps


def unshard_outputs(results, seq=S):
    out = np.empty((B, seq, HIDDEN), np.float32)
    p_attn = np.empty((B, NHEADS, seq, seq), np.float32)
    rows = seq // 4
    for c in range(NCORES):
        b, g = c // 4, c % 4
        p_attn[b, g * NH_LOC : (g + 1) * NH_LOC] = results[c]["p_out"]
        out[b, g * rows : (g + 1) * rows, :] = results[c]["out_part"]
    return out, p_attn


def kernel(query, key, value, Wq, bq, Wk, bk, Wv, bv, Wo, bo, _trace=False):
    from concourse.bass_utils import run_bass_kernel_spmd

    args = [np.asarray(a, dtype=np.float32) for a in
            (query, key, value, Wq, bq, Wk, bk, Wv, bv, Wo, bo)]
    if "nc" not in _BUILT:
        _BUILT["nc"] = build_nc(S)
    in_maps = shard_inputs(*args)
    res = run_bass_kernel_spmd(
        _BUILT["nc"], in_maps, core_ids=list(range(NCORES)), trace=_trace
    )
    out, p_attn = unshard_outputs(res.results)
    if _trace:
        kernel.last_exec_time_ns = res.exec_time_ns
        kernel.last_trace = res.instructions_and_trace
    return out, p_attn
